# revision 16
# baseline (speedup 1.0000x reference)
"""Multi-head attention (B=2, S=2048, D=2048, H=16) on 8 Trainium2 NeuronCores.

Sharding: 2-way batch x 4-way head-group tensor parallelism. Core c handles
batch c//4 and heads [4*(c%4), 4*(c%4)+4). Each core:
  - projects its 4 heads' Q^T, K^T (head-dim-major) and V (row-major) with
    bf16 matmuls (full PE rate; bf16 weight loads hide behind compute),
  - runs softmax(QK^T)V per head (1/sqrt(dk) pre-folded into Wq host-side)
    with scores kept K-major so the PV contraction needs no transposes.
    Scores land in [128, 1024] two-bank PSUM tiles so one Exp activation
    covers two K-chunks; the QK->exp->PV chain is software-pipelined two
    pair-groups deep so the scalar engine's exp latency never stalls PE,
  - contracts its 4 heads' output slice with its Wo row-slice into a partial
    [S, D] bf16 output, interleaved into the attention stream as PE filler.
Host sums the 4 partials per batch and adds bo.

All inputs are cast to bf16 on the host (halves DMA traffic; rel-err budget
is 2e-2, bf16 end-to-end lands ~6e-3). Inputs are fed pre-transposed
(q/k/v as [D, S] per batch) so every DMA is a large contiguous-run transfer
(2 KiB per partition line) and no on-chip transposes are needed anywhere.
"""

import os
import sys

for _p in ("/opt/trn_rl_repo", "/opt/pypackages"):
    if _p not in sys.path:
        sys.path.insert(0, _p)

import numpy as np
import ml_dtypes

import concourse.bass as bass
import concourse.mybir as mybir
import concourse.tile as tile
from concourse import bacc
from concourse.bass_utils import run_bass_kernel_spmd

B = 2
S = 2048
D = 2048
H = 16
DK = 128
N_CORES = 8
HPC = 4          # heads per core
CW = HPC * DK    # per-core projection width = 512
P = 128
NRB = S // 512   # 512-row blocks
NDO = D // P     # contraction chunks
NKC = S // P     # 128-row K chunks
NPR = NKC // 2   # K-chunk pairs per head
INV_SQRT_DK = 1.0 / float(np.sqrt(DK))

f32 = mybir.dt.float32
bf16 = mybir.dt.bfloat16
BF = ml_dtypes.bfloat16

_CACHE = {}
LAST_EXEC_NS = None


def _build(with_bias=True):
    nc = bacc.Bacc(None, target_bir_lowering=False, debug=False)

    qT = nc.declare_dram_parameter("qT", [D, S], bf16, isOutput=False)
    kT = nc.declare_dram_parameter("kT", [D, S], bf16, isOutput=False)
    vT = nc.declare_dram_parameter("vT", [D, S], bf16, isOutput=False)
    Wq = nc.declare_dram_parameter("Wq", [D, CW], bf16, isOutput=False)
    Wk = nc.declare_dram_parameter("Wk", [D, CW], bf16, isOutput=False)
    Wv = nc.declare_dram_parameter("Wv", [D, CW], bf16, isOutput=False)
    Wo = nc.declare_dram_parameter("Wo", [CW, D], bf16, isOutput=False)
    bq = nc.declare_dram_parameter("bq", [1, CW], bf16, isOutput=False)
    bk = nc.declare_dram_parameter("bk", [1, CW], bf16, isOutput=False)
    bv = nc.declare_dram_parameter("bv", [1, CW], bf16, isOutput=False)
    Y = nc.declare_dram_parameter("Y", [S, D], bf16, isOutput=True)

    qT3 = qT.rearrange("(do di) s -> di do s", di=P)
    kT3 = kT.rearrange("(do di) s -> di do s", di=P)
    vT3 = vT.rearrange("(do di) s -> di do s", di=P)
    Wq3 = Wq.rearrange("(do di) c -> di do c", di=P)
    Wk3 = Wk.rearrange("(do di) c -> di do c", di=P)
    Wv3 = Wv.rearrange("(do di) c -> di do c", di=P)
    Wo3 = Wo.rearrange("(cc ci) e -> ci cc e", ci=P)

    with tile.TileContext(nc) as tc:
        with (
            tc.tile_pool(name="const", bufs=1) as cp,
            tc.tile_pool(name="qkv", bufs=1) as qkvp,
            tc.tile_pool(name="wo", bufs=1) as wop,
        ):
            ones_t = cp.tile([P, 512], bf16, tag="ones_t")
            bq_t = cp.tile([1, CW], bf16, tag="bq")
            bk_t = cp.tile([1, CW], bf16, tag="bk")
            bv_t = cp.tile([1, CW], bf16, tag="bv")
            nc.gpsimd.memset(ones_t[:], 1.0)
            if with_bias:
                nc.sync.dma_start(out=bq_t[:], in_=bq[:])
                nc.sync.dma_start(out=bk_t[:], in_=bk[:])
                nc.sync.dma_start(out=bv_t[:], in_=bv[:])
            ones_row = ones_t[0:1, :]
            # Warm the scalar engine's Exp activation table while the PE is
            # busy projecting, so the first real exp doesn't eat the
            # ~1.5us ACT_TABLE_LOAD.
            warm = cp.tile([1, 2], bf16, tag="warm")
            nc.scalar.activation(
                warm[:], ones_t[0:1, 0:2], mybir.ActivationFunctionType.Exp,
            )

            # Resident per-head projected tensors (bf16).
            Qt = [qkvp.tile([P, S], bf16, tag=f"qt{h}", name=f"qt{h}") for h in range(HPC)]
            Kt = [qkvp.tile([P, S], bf16, tag=f"kt{h}", name=f"kt{h}") for h in range(HPC)]
            Vt = [qkvp.tile([P, CW], bf16, tag=f"vt{rc}", name=f"vt{rc}") for rc in range(NKC)]

            # ---- Phase P: projections -------------------------------------
            # Per projection: its W slice is loaded once and stays resident;
            # the input strips stream through as [P, 1024] tiles (one per
            # contraction chunk per 1024-row pair) released after their 8
            # matmuls. 2 KiB per partition line keeps the DMA engines at
            # full descriptor efficiency.
            wo_t = [
                wop.tile([P, D], bf16, tag=f"wo{cc}", name=f"wo{cc}")
                for cc in range(HPC)
            ]

            with (
                tc.tile_pool(name="xstrip", bufs=32) as xp,
                tc.tile_pool(name="wres", bufs=2) as wp,
                tc.tile_pool(name="pjps", bufs=8, space="PSUM") as pjps,
            ):
                for name, x3, w3, b_t in (
                    ("v", vT3, Wv3, bv_t),
                    ("k", kT3, Wk3, bk_t),
                    ("q", qT3, Wq3, bq_t),
                ):
                    if name == "q":
                        # Wo rides the DMA queue behind the k strips; it is
                        # resident well before the first out-projection filler.
                        for cc in range(HPC):
                            nc.sync.dma_start(out=wo_t[cc][:], in_=Wo3[:, cc, :])
                    wt = wp.tile([P, NDO, CW], bf16, tag="w", name=f"w_{name}")
                    for rbp in range(NRB // 2):
                        sdo = []
                        for do in range(NDO):
                            if rbp == 0 and do % 2 == 0:
                                wc = do // 2
                                nc.sync.dma_start(
                                    out=wt[:, wc * 2:(wc + 1) * 2, :],
                                    in_=w3[:, wc * 2:(wc + 1) * 2, :],
                                )
                            st = xp.tile([P, 1024], bf16, tag="strip",
                                         name=f"strip{name}{rbp}_{do}")
                            nc.sync.dma_start(
                                out=st[:],
                                in_=x3[:, do, rbp * 1024:(rbp + 1) * 1024],
                            )
                            sdo.append(st)
                        for sub in range(2):
                            rb = rbp * 2 + sub
                            rs = slice(rb * 512, (rb + 1) * 512)
                            ss = slice(sub * 512, (sub + 1) * 512)
                            ps4 = []
                            for j in range(4):
                                ps = pjps.tile([P, 512], f32, tag="pj",
                                               name=f"pj{name}{rb}_{j}")
                                if with_bias:
                                    if name in ("q", "k"):
                                        nc.tensor.matmul(
                                            ps[:], b_t[0:1, j * P:(j + 1) * P],
                                            ones_row, start=True, stop=False,
                                        )
                                    else:
                                        nc.tensor.matmul(
                                            ps[:], ones_t[0:1, 0:P], b_t[:],
                                            start=True, stop=False,
                                        )
                                ps4.append(ps)
                            for do in range(NDO):
                                for j in range(4):
                                    first = (do == 0) and not with_bias
                                    if name in ("q", "k"):
                                        nc.tensor.matmul(
                                            ps4[j][:],
                                            wt[:, do, j * P:(j + 1) * P],
                                            sdo[do][:, ss],
                                            start=first, stop=(do == NDO - 1),
                                        )
                                    else:
                                        nc.tensor.matmul(
                                            ps4[j][:],
                                            sdo[do][:, sub * 512 + j * P:
                                                    sub * 512 + (j + 1) * P],
                                            wt[:, do, :],
                                            start=first, stop=(do == NDO - 1),
                                        )
                            for j in range(4):
                                if name in ("q", "k"):
                                    dst = Qt if name == "q" else Kt
                                    nc.vector.tensor_copy(dst[j][:, rs], ps4[j][:])
                                else:
                                    nc.vector.tensor_copy(Vt[rb * 4 + j][:], ps4[j][:])

            # ---- Phase A: attention + output projection -------------------
            with (
                tc.tile_pool(name="pt", bufs=10) as ptp,
                tc.tile_pool(name="ot", bufs=2) as otp,
                tc.tile_pool(name="nrm", bufs=2) as nrmp,
                tc.tile_pool(name="ystage", bufs=2) as yp,
                tc.tile_pool(name="sps", bufs=3, space="PSUM") as sps,
                tc.tile_pool(name="ops", bufs=1, space="PSUM") as ops,
                tc.tile_pool(name="yps", bufs=1, space="PSUM") as yps,
            ):
                def emit_outproj_mm(oqb, ot_prev, slot, state, tail=False):
                    # One out-projection matmul of block `oqb`, interleaved
                    # into the attention stream as PE filler during exp waits.
                    # slot runs 0..63 across the 4 heads of the next block.
                    # The PSUM->SBUF drain runs on the otherwise-idle GpSimd
                    # engine so it never queues behind the DVE's softmax work.
                    # In the tail (post-attention) the sps pool is idle, so
                    # alternate banks with it to double-buffer the drain.
                    rc, eb, hc = slot // 16, (slot // 4) % 4, slot % 4
                    if hc == 0:
                        if tail and (slot // 4) % 2 == 1:
                            wide = sps.tile([P, 1024], f32, tag="s",
                                            name=f"ypss{oqb}_{rc}_{eb}")
                            state["ps_y"] = wide[:, 0:512]
                        else:
                            state["ps_y"] = yps.tile([P, 512], f32, tag="y",
                                                     name=f"yps{oqb}_{rc}_{eb}")
                    nc.tensor.matmul(
                        state["ps_y"][:],
                        ot_prev[hc][:, rc * P:(rc + 1) * P],
                        wo_t[hc][:, eb * 512:(eb + 1) * 512],
                        start=(hc == 0), stop=(hc == HPC - 1),
                    )
                    if hc == HPC - 1:
                        if eb == 0:
                            state["ytile"] = yp.tile([P, D], bf16, tag="y",
                                                     name=f"yt{oqb}_{rc}")
                        ydst = state["ytile"][:, eb * 512:(eb + 1) * 512]
                        if (slot // 4) % 2 == 0:
                            nc.scalar.copy(ydst, state["ps_y"][:])
                        else:
                            nc.vector.tensor_copy(ydst, state["ps_y"][:])
                        row0 = oqb * 512 + rc * P
                        if tail:
                            # Sync is idle in the tail: ship each 512-wide
                            # chunk as soon as it drains instead of waiting
                            # for the full row block.
                            nc.sync.dma_start(
                                out=Y[row0:row0 + P, eb * 512:(eb + 1) * 512],
                                in_=ydst,
                            )
                        elif eb == 3:
                            nc.sync.dma_start(
                                out=Y[row0:row0 + P, :], in_=state["ytile"][:]
                            )

                def attention_block(qb, prev_ot):
                    # Flat pair-stream over (head, pair), software-pipelined
                    # 2 deep: QK+exp of job i issue alongside PV+den of job
                    # i-2, so each exp has ~2 pair-groups of PE work to hide
                    # behind. Out-projection matmuls of the previous block
                    # fill the remaining PE slots.
                    qs = slice(qb * 512, (qb + 1) * 512)
                    state = {}
                    jobs = [(h, pr) for h in range(HPC) for pr in range(NPR)]
                    pts = {}
                    chains = {}
                    ps_os = {}
                    ot_tiles = [None] * HPC
                    slot_iter = iter(range(64)) if prev_ot is not None else None

                    def emitF():
                        if slot_iter is not None:
                            sl = next(slot_iter, None)
                            if sl is not None:
                                emit_outproj_mm(qb - 1, prev_ot, sl, state)

                    for i in range(len(jobs) + 2):
                        if i < len(jobs):
                            h, pr = jobs[i]
                            kc0 = 2 * pr
                            ps_s = sps.tile([P, 1024], f32, tag="s")
                            nc.tensor.matmul(
                                ps_s[:, 0:512],
                                Kt[h][:, kc0 * P:(kc0 + 1) * P],
                                Qt[h][:, qs], start=True, stop=True,
                            )
                            nc.tensor.matmul(
                                ps_s[:, 512:1024],
                                Kt[h][:, (kc0 + 1) * P:(kc0 + 2) * P],
                                Qt[h][:, qs], start=True, stop=True,
                            )
                            pt = ptp.tile([P, 1024], bf16, tag="pt")
                            nc.scalar.activation(
                                pt[:], ps_s[:],
                                mybir.ActivationFunctionType.Exp,
                            )
                            pts[(h, pr)] = pt
                        if i >= 2:
                            h, pr = jobs[i - 2]
                            kc0 = 2 * pr
                            pt = pts[(h, pr)]
                            if pr == 0:
                                ps_os[h] = ops.tile([P, 512], f32, tag="o",
                                                    name=f"o{qb}_{h}")
                                chains[h] = (
                                    nrmp.tile([P, 1024], bf16, tag="chA",
                                              name=f"chA{qb}_{h}"),
                                    nrmp.tile([P, 1024], bf16, tag="chB",
                                              name=f"chB{qb}_{h}"),
                                )
                            emitF()
                            nc.tensor.matmul(
                                ps_os[h][:], Vt[kc0][:, h * P:(h + 1) * P],
                                pt[:, 0:512],
                                start=(pr == 0), stop=False,
                            )
                            nc.tensor.matmul(
                                ps_os[h][:], Vt[kc0 + 1][:, h * P:(h + 1) * P],
                                pt[:, 512:1024],
                                start=False, stop=(pr == NPR - 1),
                            )
                            emitF()
                            chA, chB = chains[h]
                            if pr == 2:
                                nc.vector.tensor_add(
                                    chA[:], pts[(h, 0)][:], pts[(h, 2)][:])
                            elif pr == 3:
                                nc.vector.tensor_add(
                                    chB[:], pts[(h, 1)][:], pts[(h, 3)][:])
                            elif pr > 3:
                                ch = chA if pr % 2 == 0 else chB
                                nc.vector.tensor_add(ch[:], ch[:], pt[:])
                            if pr == NPR - 1:
                                nc.vector.tensor_add(chA[:], chA[:], chB[:])
                                ps_d = sps.tile([P, 1024], f32, tag="s",
                                                name=f"den{qb}_{h}")
                                nc.tensor.matmul(
                                    ps_d[:, 0:512], ones_t[:, 0:P],
                                    chA[:, 0:512], start=True, stop=False,
                                )
                                nc.tensor.matmul(
                                    ps_d[:, 0:512], ones_t[:, 0:P],
                                    chA[:, 512:1024], start=False, stop=True,
                                )
                                rbc = nrmp.tile([P, 512], f32, tag="rbc")
                                nc.vector.reciprocal_approx_fast(
                                    rbc[:], ps_d[:, 0:512])
                                ot = otp.tile([P, 512], bf16, tag=f"ot{h}",
                                              name=f"ot{qb}_{h}")
                                nc.vector.tensor_mul(ot[:], ps_os[h][:], rbc[:])
                                ot_tiles[h] = ot
                                for key in [k2 for k2 in pts if k2[0] == h]:
                                    del pts[key]
                    return ot_tiles

                prev_ot = None
                for qb in range(NRB):
                    prev_ot = attention_block(qb, prev_ot)
                # Tail: last block's output projection, un-interleaved.
                state = {}
                for slot in range(64):
                    emit_outproj_mm(NRB - 1, prev_ot, slot, state, tail=True)

    nc.compile()
    return nc


def kernel(q, k, v, Wq, bq, Wk, bk, Wv, bv, Wo, bo):
    global LAST_EXEC_NS
    q = np.asarray(q, dtype=np.float32)
    k = np.asarray(k, dtype=np.float32)
    v = np.asarray(v, dtype=np.float32)
    # 1/sqrt(dk) folded into Wq (and bq) so scores come out pre-scaled.
    Wq = np.asarray(Wq, dtype=np.float32) * INV_SQRT_DK
    Wk = np.asarray(Wk, dtype=np.float32)
    Wv = np.asarray(Wv, dtype=np.float32)
    Wo = np.asarray(Wo, dtype=np.float32)
    bq = np.asarray(bq, dtype=np.float32) * INV_SQRT_DK
    bk = np.asarray(bk, dtype=np.float32)
    bv = np.asarray(bv, dtype=np.float32)
    bo = np.asarray(bo, dtype=np.float32)

    with_bias = bool(np.any(bq) or np.any(bk) or np.any(bv))
    key = f"nc{int(with_bias)}"
    if key not in _CACHE:
        _CACHE[key] = _build(with_bias)
    nc = _CACHE[key]

    qTs = [np.ascontiguousarray(q[b].T.astype(BF)) for b in range(B)]
    kTs = [np.ascontiguousarray(k[b].T.astype(BF)) for b in range(B)]
    vTs = [np.ascontiguousarray(v[b].T.astype(BF)) for b in range(B)]

    in_maps = []
    for c in range(N_CORES):
        b = c // (N_CORES // B)
        g = c % (N_CORES // B)
        cs = slice(g * CW, (g + 1) * CW)
        in_maps.append({
            "qT": qTs[b], "kT": kTs[b], "vT": vTs[b],
            "Wq": np.ascontiguousarray(Wq[:, cs].astype(BF)),
            "Wk": np.ascontiguousarray(Wk[:, cs].astype(BF)),
            "Wv": np.ascontiguousarray(Wv[:, cs].astype(BF)),
            "Wo": np.ascontiguousarray(Wo[cs, :].astype(BF)),
            "bq": np.ascontiguousarray(bq[cs].astype(BF))[None, :],
            "bk": np.ascontiguousarray(bk[cs].astype(BF))[None, :],
            "bv": np.ascontiguousarray(bv[cs].astype(BF))[None, :],
        })

    _CACHE["last_in_maps"] = in_maps
    res = None
    for attempt in range(3):
        try:
            res = run_bass_kernel_spmd(nc, in_maps, core_ids=list(range(N_CORES)))
            break
        except Exception:
            if attempt == 2:
                raise
    LAST_EXEC_NS = res.exec_time_ns

    gpb = N_CORES // B
    out = np.empty((B, S, D), dtype=np.float32)
    for b in range(B):
        acc = res.results[b * gpb]["Y"].astype(np.float32)
        for g in range(1, gpb):
            acc = acc + res.results[b * gpb + g]["Y"].astype(np.float32)
        out[b] = acc + bo[None, :]
    return out
